# revision 14
# baseline (speedup 1.0000x reference)
"""Trainium2 Bass kernel for DPAttention (attention block + residual + LayerNorm).

Sharding: 8 cores = DP2 (batch) x TP4 (head groups of 3 heads).
Core c: b = c//4, g = c%4 -> heads [3g, 3g+3), output rows [512g, 512g+512) of batch b.

Per-core dataflow:
  X^T (f32, SBUF)
    -> Q^T/K^T [d, s] via matmul(lhsT=Wq_slice, rhs=X^T)   (heads 0,1 stacked on 128
       partitions; head 2 in its own 64-partition tile)
    -> V [s, d] via matmul(lhsT=X^T tile, rhs=Wv_slice), stored bf16 with a ones
       column appended per head (gives the softmax denominator for free)
  scores^T [k, q] = matmul(lhsT=K^T tile, rhs=Q^T chunk)   (heads 0/1 run in different
       PE row-groups concurrently)
  e = exp(scores^T * 1/8 + mask_bias_k)  on ScalarE, psum->sbuf bf16
  ctx^T [d+1, q] = sum_k matmul(lhsT=[V|1], rhs=e) ; + rank-1 (1e18 * u) matmul that
       overwrites invalid-query columns with the uniform-attention value u = mean_k V
  normalize by denominator row, write ctx^T bf16
  4-core AllGather of ctx^T -> full [768, 2048]; dynamic-offset DMA reads this core's
       512 query columns -> out dense (bf16) + residual + LayerNorm -> out [512, 768].
"""
import numpy as np
import ml_dtypes

import concourse.bass as bass
import concourse.mybir as mybir
import concourse.tile as tile
from concourse import bacc
from concourse.bass_utils import run_bass_kernel_spmd

F32 = mybir.dt.float32
BF16 = mybir.dt.bfloat16
U32 = mybir.dt.uint32
AF = mybir.ActivationFunctionType
ALU = mybir.AluOpType
AX = mybir.AxisListType

B, S, H, NH, HD = 2, 2048, 768, 12, 64
P = 128
KT = H // P            # 6 contraction tiles over hidden
ST = S // P            # 16 tiles over sequence
TP = 4                 # head groups (tensor-parallel within a batch)
HG = NH // TP          # 3 heads per core
HGD = HG * HD          # 192
SQ = S // TP           # 512 output rows per core
EPS = 1e-5
SCALE = 1.0 / np.sqrt(HD)
NCORES = 8
GROUPS = [[0, 1, 2, 3], [4, 5, 6, 7]]
BIGNEG = -1.0e9
BIGPOS = 1.0e18

_cache = {}

import os
STAGE = int(os.environ.get("KSTAGE", "6"))


def build():
    nc = bacc.Bacc(num_devices=NCORES)

    xt_d = nc.dram_tensor("xt", [H, S], F32, kind="ExternalInput")
    xres_d = nc.dram_tensor("xres", [SQ, H], F32, kind="ExternalInput")
    wq_d = nc.dram_tensor("wq", [H, HGD], F32, kind="ExternalInput")
    wk_d = nc.dram_tensor("wk", [H, HGD], F32, kind="ExternalInput")
    wv_d = nc.dram_tensor("wv", [H, HGD], F32, kind="ExternalInput")
    bq_d = nc.dram_tensor("bq", [HGD], F32, kind="ExternalInput")
    bk_d = nc.dram_tensor("bk", [HGD], F32, kind="ExternalInput")
    bvr_d = nc.dram_tensor("bvr", [P, HGD], F32, kind="ExternalInput")
    wo_d = nc.dram_tensor("wo", [H, H], BF16, kind="ExternalInput")
    mkb_d = nc.dram_tensor("mkb", [S], F32, kind="ExternalInput")
    gq_d = nc.dram_tensor("gq", [1, S], BF16, kind="ExternalInput")
    lng_d = nc.dram_tensor("lng", [P, H], F32, kind="ExternalInput")
    lnb_d = nc.dram_tensor("lnb", [P, H], F32, kind="ExternalInput")
    qoff_d = nc.dram_tensor("qoff", [1, 1], U32, kind="ExternalInput")
    out_d = nc.dram_tensor("out", [SQ, H], F32, kind="ExternalOutput")

    with tile.TileContext(nc) as tc:
        with (
            tc.tile_pool(name="wts", bufs=1) as wts,
            tc.tile_pool(name="qkv", bufs=1) as qkv,
            tc.tile_pool(name="dram", bufs=1, space="DRAM") as dram,
        ):
            # ---- load weights / small tensors ----
            wq_sb = wts.tile([P, KT, HGD], F32)
            wk_sb = wts.tile([P, KT, HGD], F32)
            wv_sb = wts.tile([P, KT, HGD], F32)
            nc.sync.dma_start(wq_sb[:], wq_d.rearrange("(kt p) d -> p kt d", p=P))
            nc.sync.dma_start(wk_sb[:], wk_d.rearrange("(kt p) d -> p kt d", p=P))
            nc.sync.dma_start(wv_sb[:], wv_d.rearrange("(kt p) d -> p kt d", p=P))
            wo_sb = wts.tile([P, KT, H], BF16)
            nc.sync.dma_start(wo_sb[:], wo_d.rearrange("(kt p) n -> p kt n", p=P))

            bq_sb = wts.tile([P, 2], F32)
            bk_sb = wts.tile([P, 2], F32)
            nc.gpsimd.dma_start(bq_sb[:, 0:1], bq_d[0:P].rearrange("(p o) -> p o", o=1))
            nc.gpsimd.dma_start(bq_sb[0:HGD - P, 1:2], bq_d[P:HGD].rearrange("(p o) -> p o", o=1))
            nc.gpsimd.dma_start(bk_sb[:, 0:1], bk_d[0:P].rearrange("(p o) -> p o", o=1))
            nc.gpsimd.dma_start(bk_sb[0:HGD - P, 1:2], bk_d[P:HGD].rearrange("(p o) -> p o", o=1))
            bvr_sb = wts.tile([P, HG, HD], F32)
            nc.sync.dma_start(bvr_sb[:], bvr_d.rearrange("p (h d) -> p h d", d=HD))
            mkb_sb = wts.tile([P, ST], F32)
            nc.gpsimd.dma_start(mkb_sb[:], mkb_d.rearrange("(kt p) -> p kt", p=P))
            gq_sb = wts.tile([1, S], BF16)
            nc.gpsimd.dma_start(gq_sb[:], gq_d[:])
            lng_sb = wts.tile([P, H], F32)
            lnb_sb = wts.tile([P, H], F32)
            nc.sync.dma_start(lng_sb[:], lng_d[:])
            nc.sync.dma_start(lnb_sb[:], lnb_d[:])
            xres_sb = wts.tile([P, SQ // P, H], F32)
            nc.sync.dma_start(xres_sb[:], xres_d.rearrange("(t p) n -> p t n", p=P))
            qoff_sb = wts.tile([1, 1], U32)
            nc.gpsimd.dma_start(qoff_sb[:], qoff_d[:])

            ones_sb = wts.tile([P, 1], BF16)
            nc.gpsimd.memset(ones_sb[:], 1.0)

            # ---- persistent intermediate tiles ----
            qt_sb = qkv.tile([P, S], F32)      # Q^T heads 0,1 (rows 0:64 / 64:128)
            kt_sb = qkv.tile([P, S], F32)
            qt2_sb = qkv.tile([HD, S], F32)    # Q^T head 2
            kt2_sb = qkv.tile([HD, S], F32)
            v_sb = qkv.tile([P, ST, HG, HD + 1], BF16)   # V + ones column per head
            u_sb = qkv.tile([1, HG, HD + 1], BF16)       # mean_k V (+1 slot) per head
            ctxa_sb = qkv.tile([P, S], BF16)   # ctx^T heads 0,1
            ctxb_sb = qkv.tile([HD, S], BF16)  # ctx^T head 2

            # ================= projections =================
            with tc.tile_pool(name="xt", bufs=1) as xtp, \
                 tc.tile_pool(name="pps", bufs=3, space="PSUM") as pps, \
                 tc.tile_pool(name="vps", bufs=2, space="PSUM") as vps:
                xt_sb = xtp.tile([P, KT, S], F32)
                nc.sync.dma_start(xt_sb[:], xt_d.rearrange("(kt p) s -> p kt s", p=P))

                # Q^T / K^T: two M-passes (128 for heads 0,1; 64 for head 2)
                for w_sb, b_sb, dst, dst2 in (
                    (wq_sb, bq_sb, qt_sb, qt2_sb),
                    (wk_sb, bk_sb, kt_sb, kt2_sb),
                ):
                    for mp, (m0, msz, d_sb) in enumerate(((0, P, dst), (P, HGD - P, dst2))):
                        for qc in range(S // 512):
                            ps = pps.tile([P, 512], F32, tag="proj")
                            for kt in range(KT):
                                nc.tensor.matmul(
                                    ps[:msz],
                                    w_sb[:, kt, m0:m0 + msz],
                                    xt_sb[:, kt, qc * 512:(qc + 1) * 512],
                                    start=(kt == 0), stop=(kt == KT - 1),
                                )
                            nc.vector.tensor_scalar_add(
                                d_sb[:msz, qc * 512:(qc + 1) * 512], ps[:msz],
                                b_sb[:msz, mp:mp + 1],
                            )

                # V in [s, d] layout (+ bias broadcast along partitions from host)
                for st in range(ST):
                    ps = vps.tile([P, HGD], F32, tag="vproj")
                    for kt in range(KT):
                        nc.tensor.matmul(
                            ps[:], xt_sb[:, kt, st * P:(st + 1) * P], wv_sb[:, kt, :],
                            start=(kt == 0), stop=(kt == KT - 1),
                        )
                    nc.vector.tensor_tensor(
                        v_sb[:, st, :, 0:HD], ps[:].rearrange("p (h d) -> p h d", d=HD),
                        bvr_sb[:], op=ALU.add,
                    )
                nc.gpsimd.memset(v_sb[:, :, :, HD:HD + 1], 1.0)

                # u = mean_k V per head
                ups = vps.tile([1, HGD], F32, tag="u")
                for st in range(ST):
                    nc.tensor.matmul(
                        ups[:], ones_sb[:], v_sb[:, st, :, 0:HD],
                        start=(st == 0), stop=(st == ST - 1),
                    )
                nc.vector.tensor_scalar_mul(
                    u_sb[0:1, :, 0:HD], ups[:].rearrange("p (h d) -> p h d", d=HD), 1.0 / S)
                nc.gpsimd.memset(u_sb[:, :, HD:HD + 1], 1.0)

            # ================= attention (software-pipelined units) =================
            QH = S // 1024  # 2 query halves per head
            units = [(h, qh) for h in range(HG) for qh in range(QH)] if STAGE >= 2 else []

            with tc.tile_pool(name="epool", bufs=2) as epool, \
                 tc.tile_pool(name="sps", bufs=2, space="PSUM") as sps, \
                 tc.tile_pool(name="cps", bufs=2, space="PSUM") as cps, \
                 tc.tile_pool(name="npool", bufs=2) as npool:

                e_tiles = {}
                c_tiles = {}

                def emit_scores_kt(i, kt):
                    h, qh = units[i]
                    e_t = e_tiles[i]
                    ps = sps.tile([P, 1024], F32, tag="sc")
                    for sub in range(2):
                        q0 = qh * 1024 + sub * 512
                        if h < 2:
                            lhsT = kt_sb[HD * h:HD * (h + 1), kt * P:(kt + 1) * P]
                            rhs = qt_sb[HD * h:HD * (h + 1), q0:q0 + 512]
                        else:
                            lhsT = kt2_sb[:, kt * P:(kt + 1) * P]
                            rhs = qt2_sb[:, q0:q0 + 512]
                        nc.tensor.matmul(ps[:, sub * 512:(sub + 1) * 512], lhsT, rhs,
                                         start=True, stop=True)
                    nc.scalar.activation(e_t[:, kt, :], ps[:], AF.Exp,
                                         bias=mkb_sb[:, kt:kt + 1], scale=float(SCALE))

                def emit_ctx_kt(i, kt):
                    h, qh = units[i]
                    e_t = e_tiles[i]
                    pa, pb = c_tiles[i]
                    for sub, pc in ((0, pa), (1, pb)):
                        nc.tensor.matmul(
                            pc[:], v_sb[:, kt, h, :], e_t[:, kt, sub * 512:(sub + 1) * 512],
                            start=(kt == 0), stop=False,
                        )

                def emit_ctx_tail(i):
                    h, qh = units[i]
                    pa, pb = c_tiles[i]
                    recip = npool.tile([1, 1024], F32, tag="recip")
                    for sub, pc in ((0, pa), (1, pb)):
                        q0 = qh * 1024 + sub * 512
                        nc.tensor.matmul(pc[:], u_sb[0:1, h, :], gq_sb[0:1, q0:q0 + 512],
                                         start=False, stop=True)
                        nc.vector.reciprocal(recip[:, sub * 512:(sub + 1) * 512],
                                             pc[HD:HD + 1, :])
                    rden = dram.tile([1, 1024], F32, tag="rden", bufs=2,
                                     name=f"rden{i}")
                    nc.sync.dma_start(rden[:], recip[:])
                    rb = npool.tile([HD, 1024], F32, tag="rb")
                    nc.sync.dma_start(rb[:], rden[0:1, :].to_broadcast((HD, 1024)))
                    for sub, pc in ((0, pa), (1, pb)):
                        q0 = qh * 1024 + sub * 512
                        dst = (ctxa_sb[HD * h:HD * (h + 1), q0:q0 + 512] if h < 2
                               else ctxb_sb[:, q0:q0 + 512])
                        nc.vector.tensor_tensor(dst, pc[0:HD, :],
                                                rb[:, sub * 512:(sub + 1) * 512],
                                                op=ALU.mult)

                for i in range(len(units) + 1):
                    if i < len(units):
                        e_tiles[i] = epool.tile([P, ST, 1024], BF16, tag="e", name=f"e{i}")
                        c_tiles[i] = (cps.tile([HD + 1, 512], F32, tag="ca", name=f"ca{i}"),
                                      cps.tile([HD + 1, 512], F32, tag="cb", name=f"cb{i}"))
                    for kt in range(ST):
                        if i < len(units):
                            emit_scores_kt(i, kt)
                        if i > 0:
                            emit_ctx_kt(i - 1, kt)
                    if i > 0:
                        emit_ctx_tail(i - 1)
                        del e_tiles[i - 1]

            # ================= gather ctx across the TP group =================
            if STAGE >= 3:
                ag_in = dram.tile([HGD, S], BF16)
                nc.sync.dma_start(ag_in[0:P, :], ctxa_sb[:])
                nc.sync.dma_start(ag_in[P:HGD, :], ctxb_sb[:])
                ag_out = dram.tile([TP, HGD, S], BF16)
                nc.gpsimd.collective_compute(
                    "AllGather", ALU.bypass, replica_groups=GROUPS,
                    ins=[ag_in.opt()], outs=[ag_out.opt()],
                )

                with tc.tile_critical():
                    with nc.gpsimd.register("qo") as qo_reg:
                        nc.gpsimd.reg_load(qo_reg, qoff_sb[0:1, 0:1])
                        qoff_v = nc.gpsimd.snap(qo_reg)

                ctxg_sb = qkv.tile([P, KT, SQ], BF16)
                nc.gpsimd.dma_start(
                    ctxg_sb[:],
                    ag_out.rearrange("g d q -> (g d) q").rearrange("(kt p) q -> p kt q", p=P)[
                        :, :, bass.ds(qoff_v, SQ)],
                )

            # ================= out dense + residual + LayerNorm =================
            if STAGE < 4:
                with tc.tile_pool(name="dummy", bufs=1) as dpool:
                    for st4 in range(SQ // P):
                        d_sb = dpool.tile([P, H], F32, tag="d")
                        nc.vector.tensor_copy(d_sb[:], xres_sb[:, st4, :])
                        nc.sync.dma_start(out_d[st4 * P:(st4 + 1) * P, :], d_sb[:])
            with tc.tile_pool(name="ops", bufs=2, space="PSUM") as ops, \
                 tc.tile_pool(name="lnp", bufs=2) as lnp:
                for st4 in range(SQ // P if STAGE >= 4 else 0):
                    ps = ops.tile([P, H], F32, tag="od")
                    for kt in range(KT):
                        lhsT = ctxg_sb[:, kt, st4 * P:(st4 + 1) * P]
                        nc.tensor.matmul(ps[:, 0:512], lhsT, wo_sb[:, kt, 0:512],
                                         start=(kt == 0), stop=(kt == KT - 1))
                        nc.tensor.matmul(ps[:, 512:H], lhsT, wo_sb[:, kt, 512:H],
                                         start=(kt == 0), stop=(kt == KT - 1))
                    h_sb = lnp.tile([P, H], F32, tag="h")
                    nc.vector.tensor_tensor(h_sb[:], ps[:], xres_sb[:, st4, :], op=ALU.add)

                    if STAGE < 5:
                        nc.sync.dma_start(out_d[st4 * P:(st4 + 1) * P, :], h_sb[:])
                        continue

                    mu = lnp.tile([P, 1], F32, tag="mu")
                    nc.vector.reduce_sum(mu[:], h_sb[:], axis=AX.X)
                    nc.vector.tensor_scalar_mul(mu[:], mu[:], 1.0 / H)
                    hc = lnp.tile([P, H], F32, tag="hc")
                    nc.vector.tensor_scalar_sub(hc[:], h_sb[:], mu[:])
                    sq = lnp.tile([P, H], F32, tag="sq")
                    var = lnp.tile([P, 1], F32, tag="var")
                    if STAGE >= 7:
                        nc.vector.tensor_tensor_reduce(
                            out=sq[:], in0=hc[:], in1=hc[:], scale=1.0, scalar=0.0,
                            op0=ALU.mult, op1=ALU.add, accum_out=var[:])
                    else:
                        nc.vector.tensor_mul(sq[:], hc[:], hc[:])
                        nc.vector.reduce_sum(var[:], sq[:], axis=AX.X)
                    nc.vector.tensor_scalar_mul(var[:], var[:], 1.0 / H)
                    nc.vector.tensor_scalar_add(var[:], var[:], EPS)
                    # rstd = 1/sqrt(var), with one Newton step to fix ACT sqrt error
                    std0 = lnp.tile([P, 1], F32, tag="std0")
                    if STAGE >= 6:
                        nc.scalar.activation(std0[:], var[:], AF.Sqrt)
                    else:
                        nc.vector.tensor_copy(std0[:], var[:])
                    y0 = lnp.tile([P, 1], F32, tag="y0")
                    nc.vector.reciprocal(y0[:], std0[:])
                    t0 = lnp.tile([P, 1], F32, tag="t0")
                    nc.vector.tensor_tensor(t0[:], y0[:], y0[:], op=ALU.mult)
                    nc.vector.tensor_tensor(t0[:], t0[:], var[:], op=ALU.mult)
                    nc.vector.tensor_scalar_mul(t0[:], t0[:], -0.5)
                    nc.vector.tensor_scalar_add(t0[:], t0[:], 1.5)
                    rstd = lnp.tile([P, 1], F32, tag="rstd")
                    nc.vector.tensor_tensor(rstd[:], y0[:], t0[:], op=ALU.mult)

                    o_sb = lnp.tile([P, H], F32, tag="o")
                    nc.vector.tensor_scalar_mul(o_sb[:], hc[:], rstd[:])
                    nc.vector.tensor_tensor(o_sb[:], o_sb[:], lng_sb[:], op=ALU.mult)
                    nc.vector.tensor_tensor(o_sb[:], o_sb[:], lnb_sb[:], op=ALU.add)
                    nc.sync.dma_start(out_d[st4 * P:(st4 + 1) * P, :], o_sb[:])

    nc.compile()
    return nc


def _prep_inputs(inputs):
    hs = np.asarray(inputs["hidden_states"], dtype=np.float32)
    am = np.asarray(inputs["attention_mask"], dtype=np.float32)
    Wq = np.asarray(inputs["Wq"], dtype=np.float32)
    Wk = np.asarray(inputs["Wk"], dtype=np.float32)
    Wv = np.asarray(inputs["Wv"], dtype=np.float32)
    Wo = np.asarray(inputs["Wo"], dtype=np.float32)
    bq = np.asarray(inputs["bq"], dtype=np.float32)
    bk = np.asarray(inputs["bk"], dtype=np.float32)
    bv = np.asarray(inputs["bv"], dtype=np.float32)
    bo = np.asarray(inputs["bo"], dtype=np.float32)
    lng = np.asarray(inputs["ln_gamma"], dtype=np.float32)
    lnb = np.asarray(inputs["ln_beta"], dtype=np.float32)

    wo_bf = Wo.astype(ml_dtypes.bfloat16)
    lng_rep = np.ascontiguousarray(np.broadcast_to(lng, (P, H)))
    lnb_rep = np.ascontiguousarray(np.broadcast_to(lnb, (P, H)))

    in_maps = []
    for c in range(NCORES):
        b, g = c // TP, c % TP
        cs = slice(HGD * g, HGD * (g + 1))
        mk = np.where(am[b] >= 0, 0.0, BIGNEG).astype(np.float32)
        gqv = np.where(am[b] >= 0, 0.0, BIGPOS).astype(ml_dtypes.bfloat16)[None, :]
        in_maps.append({
            "xt": np.ascontiguousarray(hs[b].T),
            "xres": np.ascontiguousarray(hs[b, SQ * g:SQ * (g + 1)] + bo),
            "wq": np.ascontiguousarray(Wq[:, cs]),
            "wk": np.ascontiguousarray(Wk[:, cs]),
            "wv": np.ascontiguousarray(Wv[:, cs]),
            "bq": np.ascontiguousarray(bq[cs]),
            "bk": np.ascontiguousarray(bk[cs]),
            "bvr": np.ascontiguousarray(np.broadcast_to(bv[cs], (P, HGD))),
            "wo": np.ascontiguousarray(wo_bf),
            "mkb": mk,
            "gq": np.ascontiguousarray(gqv),
            "lng": lng_rep,
            "lnb": lnb_rep,
            "qoff": np.array([[SQ * g]], dtype=np.uint32),
        })
    return in_maps


def _run(inputs, trace=False, trace_cores=None):
    if "nc" not in _cache:
        _cache["nc"] = build()
    nc = _cache["nc"]
    in_maps = _prep_inputs(inputs)
    res = run_bass_kernel_spmd(
        nc, in_maps, list(range(NCORES)), trace=trace,
        trace_cores=trace_cores,
    )
    out = np.empty((B, S, H), dtype=np.float32)
    for c in range(NCORES):
        b, g = c // TP, c % TP
        out[b, SQ * g:SQ * (g + 1)] = res.results[c]["out"]
    return out, res


def kernel(**inputs) -> np.ndarray:
    out, _ = _run(inputs)
    return out


# revision 17
# speedup vs baseline: 1.8734x; 1.8734x over previous
"""Trainium2 Bass kernel for DPAttention (attention block + residual + LayerNorm).

Sharding: 8 cores = DP2 (batch) x TP4 (head groups of 3 heads).
Core c: b = c//4, g = c%4 -> heads [3g, 3g+3), output rows [512g, 512g+512) of batch b.

Per-core dataflow:
  X^T (f32, SBUF)
    -> Q^T/K^T [d, s] via matmul(lhsT=Wq_slice, rhs=X^T)   (heads 0,1 stacked on 128
       partitions; head 2 in its own 64-partition tile)
    -> V [s, d] via matmul(lhsT=X^T tile, rhs=Wv_slice), stored bf16 with a ones
       column appended per head (gives the softmax denominator for free)
  scores^T [k, q] = matmul(lhsT=K^T tile, rhs=Q^T chunk)   (heads 0/1 run in different
       PE row-groups concurrently)
  e = exp(scores^T * 1/8 + mask_bias_k)  on ScalarE, psum->sbuf bf16
  ctx^T [d+1, q] = sum_k matmul(lhsT=[V|1], rhs=e) ; + rank-1 (1e18 * u) matmul that
       overwrites invalid-query columns with the uniform-attention value u = mean_k V
  normalize by denominator row, write ctx^T bf16
  4-core AllGather of ctx^T -> full [768, 2048]; dynamic-offset DMA reads this core's
       512 query columns -> out dense (bf16) + residual + LayerNorm -> out [512, 768].
"""
import numpy as np
import ml_dtypes

import concourse.bass as bass
import concourse.mybir as mybir
import concourse.tile as tile
from concourse import bacc
from concourse.bass_utils import run_bass_kernel_spmd

F32 = mybir.dt.float32
BF16 = mybir.dt.bfloat16
U32 = mybir.dt.uint32
AF = mybir.ActivationFunctionType
ALU = mybir.AluOpType
AX = mybir.AxisListType

B, S, H, NH, HD = 2, 2048, 768, 12, 64
P = 128
KT = H // P            # 6 contraction tiles over hidden
ST = S // P            # 16 tiles over sequence
TP = 4                 # head groups (tensor-parallel within a batch)
HG = NH // TP          # 3 heads per core
HGD = HG * HD          # 192
SQ = S // TP           # 512 output rows per core
EPS = 1e-5
SCALE = 1.0 / np.sqrt(HD)
NCORES = 8
GROUPS = [[0, 1, 2, 3], [4, 5, 6, 7]]
BIGNEG = -1.0e9
BIGPOS = 1.0e18

_cache = {}

import os
STAGE = int(os.environ.get("KSTAGE", "6"))


def build():
    nc = bacc.Bacc(num_devices=NCORES)

    xt_d = nc.dram_tensor("xt", [H, S], BF16, kind="ExternalInput")
    xres_d = nc.dram_tensor("xres", [SQ, H], F32, kind="ExternalInput")
    wq_d = nc.dram_tensor("wq", [H, HGD], BF16, kind="ExternalInput")
    wk_d = nc.dram_tensor("wk", [H, HGD], BF16, kind="ExternalInput")
    wv_d = nc.dram_tensor("wv", [H, HGD], BF16, kind="ExternalInput")
    bq_d = nc.dram_tensor("bq", [HGD], F32, kind="ExternalInput")
    bk_d = nc.dram_tensor("bk", [HGD], F32, kind="ExternalInput")
    bvr_d = nc.dram_tensor("bvr", [P, HGD], F32, kind="ExternalInput")
    wo_d = nc.dram_tensor("wo", [H, H], BF16, kind="ExternalInput")
    mkb_d = nc.dram_tensor("mkb", [S], F32, kind="ExternalInput")
    gq_d = nc.dram_tensor("gq", [1, S], BF16, kind="ExternalInput")
    lng_d = nc.dram_tensor("lng", [P, H], F32, kind="ExternalInput")
    lnb_d = nc.dram_tensor("lnb", [P, H], F32, kind="ExternalInput")
    qoff_d = nc.dram_tensor("qoff", [1, 1], U32, kind="ExternalInput")
    out_d = nc.dram_tensor("out", [SQ, H], F32, kind="ExternalOutput")

    with tile.TileContext(nc) as tc:
        with (
            tc.tile_pool(name="wts", bufs=1) as wts,
            tc.tile_pool(name="qkv", bufs=1) as qkv,
            tc.tile_pool(name="dram", bufs=1, space="DRAM") as dram,
        ):
            # ---- load weights / small tensors ----
            wq_sb = wts.tile([P, KT, HGD], BF16)
            wk_sb = wts.tile([P, KT, HGD], BF16)
            wv_sb = wts.tile([P, KT, HGD], BF16)
            nc.sync.dma_start(wq_sb[:], wq_d.rearrange("(kt p) d -> p kt d", p=P))
            nc.sync.dma_start(wk_sb[:], wk_d.rearrange("(kt p) d -> p kt d", p=P))
            nc.sync.dma_start(wv_sb[:], wv_d.rearrange("(kt p) d -> p kt d", p=P))
            wo_sb = wts.tile([P, KT, H], BF16)
            nc.sync.dma_start(wo_sb[:], wo_d.rearrange("(kt p) n -> p kt n", p=P))

            bq_sb = wts.tile([P, 2], F32)
            bk_sb = wts.tile([P, 2], F32)
            nc.gpsimd.dma_start(bq_sb[:, 0:1], bq_d[0:P].rearrange("(p o) -> p o", o=1))
            nc.gpsimd.dma_start(bq_sb[0:HGD - P, 1:2], bq_d[P:HGD].rearrange("(p o) -> p o", o=1))
            nc.gpsimd.dma_start(bk_sb[:, 0:1], bk_d[0:P].rearrange("(p o) -> p o", o=1))
            nc.gpsimd.dma_start(bk_sb[0:HGD - P, 1:2], bk_d[P:HGD].rearrange("(p o) -> p o", o=1))
            bvr_sb = wts.tile([P, HG, HD], F32)
            nc.sync.dma_start(bvr_sb[:], bvr_d.rearrange("p (h d) -> p h d", d=HD))
            mkb_sb = wts.tile([P, ST], F32)
            nc.gpsimd.dma_start(mkb_sb[:], mkb_d.rearrange("(kt p) -> p kt", p=P))
            gq_sb = wts.tile([1, S], BF16)
            nc.gpsimd.dma_start(gq_sb[:], gq_d[:])
            lng_sb = wts.tile([P, H], F32)
            lnb_sb = wts.tile([P, H], F32)
            nc.sync.dma_start(lng_sb[:], lng_d[:])
            nc.sync.dma_start(lnb_sb[:], lnb_d[:])
            xres_sb = wts.tile([P, SQ // P, H], F32)
            nc.sync.dma_start(xres_sb[:], xres_d.rearrange("(t p) n -> p t n", p=P))
            qoff_sb = wts.tile([1, 1], U32)
            nc.gpsimd.dma_start(qoff_sb[:], qoff_d[:])

            ones_sb = wts.tile([P, 1], BF16)
            nc.gpsimd.memset(ones_sb[:], 1.0)

            # ---- persistent intermediate tiles ----
            qt_sb = qkv.tile([P, S], BF16)      # Q^T heads 0,1 (rows 0:64 / 64:128)
            kt_sb = qkv.tile([P, S], BF16)
            qt2_sb = qkv.tile([HD, S], BF16)    # Q^T head 2
            kt2_sb = qkv.tile([HD, S], BF16)
            v_sb = qkv.tile([P, ST, HG, HD + 1], BF16)   # V + ones column per head
            u_sb = qkv.tile([1, HG, HD + 1], BF16)       # mean_k V (+1 slot) per head
            ctxa_sb = qkv.tile([P, S], BF16)   # ctx^T heads 0,1
            ctxb_sb = qkv.tile([HD, S], BF16)  # ctx^T head 2

            # ================= projections =================
            with tc.tile_pool(name="xt", bufs=1) as xtp, \
                 tc.tile_pool(name="pps", bufs=3, space="PSUM") as pps, \
                 tc.tile_pool(name="vps", bufs=2, space="PSUM") as vps:
                xt_sb = xtp.tile([P, KT, S], BF16)
                nc.sync.dma_start(xt_sb[:], xt_d.rearrange("(kt p) s -> p kt s", p=P))

                # Q^T / K^T: two M-passes (128 for heads 0,1; 64 for head 2)
                for w_sb, b_sb, dst, dst2 in (
                    (wq_sb, bq_sb, qt_sb, qt2_sb),
                    (wk_sb, bk_sb, kt_sb, kt2_sb),
                ):
                    for mp, (m0, msz, d_sb) in enumerate(((0, P, dst), (P, HGD - P, dst2))):
                        for qc in range(S // 512):
                            ps = pps.tile([P, 512], F32, tag="proj")
                            for kt in range(KT):
                                nc.tensor.matmul(
                                    ps[:msz],
                                    w_sb[:, kt, m0:m0 + msz],
                                    xt_sb[:, kt, qc * 512:(qc + 1) * 512],
                                    start=(kt == 0), stop=(kt == KT - 1),
                                )
                            nc.vector.tensor_scalar_add(
                                d_sb[:msz, qc * 512:(qc + 1) * 512], ps[:msz],
                                b_sb[:msz, mp:mp + 1],
                            )

                # V in [s, d] layout (+ bias broadcast along partitions from host)
                for st in range(ST):
                    ps = vps.tile([P, HGD], F32, tag="vproj")
                    for kt in range(KT):
                        nc.tensor.matmul(
                            ps[:], xt_sb[:, kt, st * P:(st + 1) * P], wv_sb[:, kt, :],
                            start=(kt == 0), stop=(kt == KT - 1),
                        )
                    nc.vector.tensor_tensor(
                        v_sb[:, st, :, 0:HD], ps[:].rearrange("p (h d) -> p h d", d=HD),
                        bvr_sb[:], op=ALU.add,
                    )
                nc.gpsimd.memset(v_sb[:, :, :, HD:HD + 1], 1.0)

                # u = mean_k V per head
                ups = vps.tile([1, HGD], F32, tag="u")
                for st in range(ST):
                    nc.tensor.matmul(
                        ups[:], ones_sb[:], v_sb[:, st, :, 0:HD],
                        start=(st == 0), stop=(st == ST - 1),
                    )
                nc.vector.tensor_scalar_mul(
                    u_sb[0:1, :, 0:HD], ups[:].rearrange("p (h d) -> p h d", d=HD), 1.0 / S)
                nc.gpsimd.memset(u_sb[:, :, HD:HD + 1], 1.0)

            # ================= attention (software-pipelined units) =================
            QH = S // 1024  # 2 query halves per head
            units = [(h, qh) for h in range(HG) for qh in range(QH)] if STAGE >= 2 else []

            with tc.tile_pool(name="epool", bufs=2) as epool, \
                 tc.tile_pool(name="sps", bufs=2, space="PSUM") as sps, \
                 tc.tile_pool(name="cps", bufs=2, space="PSUM") as cps, \
                 tc.tile_pool(name="npool", bufs=2) as npool:

                e_tiles = {}
                c_tiles = {}

                def emit_scores_kt(i, kt):
                    h, qh = units[i]
                    e_t = e_tiles[i]
                    ps = sps.tile([P, 1024], F32, tag="sc")
                    for sub in range(2):
                        q0 = qh * 1024 + sub * 512
                        if h < 2:
                            lhsT = kt_sb[HD * h:HD * (h + 1), kt * P:(kt + 1) * P]
                            rhs = qt_sb[HD * h:HD * (h + 1), q0:q0 + 512]
                        else:
                            lhsT = kt2_sb[:, kt * P:(kt + 1) * P]
                            rhs = qt2_sb[:, q0:q0 + 512]
                        nc.tensor.matmul(ps[:, sub * 512:(sub + 1) * 512], lhsT, rhs,
                                         start=True, stop=True)
                    nc.scalar.activation(e_t[:, kt, :], ps[:], AF.Exp,
                                         bias=mkb_sb[:, kt:kt + 1], scale=float(SCALE))

                def emit_ctx_kt(i, kt):
                    h, qh = units[i]
                    e_t = e_tiles[i]
                    pa, pb = c_tiles[i]
                    for sub, pc in ((0, pa), (1, pb)):
                        nc.tensor.matmul(
                            pc[:], v_sb[:, kt, h, :], e_t[:, kt, sub * 512:(sub + 1) * 512],
                            start=(kt == 0), stop=False,
                        )

                def emit_ctx_tail(i):
                    h, qh = units[i]
                    pa, pb = c_tiles[i]
                    rden = dram.tile([1, 1024], F32, tag="rden", bufs=2,
                                     name=f"rden{i}")
                    den = npool.tile([1, 1024], F32, tag="den")
                    for sub, pc in ((0, pa), (1, pb)):
                        q0 = qh * 1024 + sub * 512
                        nc.tensor.matmul(pc[:], u_sb[0:1, h, :], gq_sb[0:1, q0:q0 + 512],
                                         start=False, stop=True)
                        nc.vector.tensor_copy(den[:, sub * 512:(sub + 1) * 512],
                                              pc[HD:HD + 1, :])
                    nc.sync.dma_start(rden[:], den[:])
                    rb = npool.tile([HD, 1024], F32, tag="rb")
                    nc.sync.dma_start(rb[:], rden[0:1, :].to_broadcast((HD, 1024)))
                    nc.vector.reciprocal(rb[:], rb[:])
                    for sub, pc in ((0, pa), (1, pb)):
                        q0 = qh * 1024 + sub * 512
                        dst = (ctxa_sb[HD * h:HD * (h + 1), q0:q0 + 512] if h < 2
                               else ctxb_sb[:, q0:q0 + 512])
                        nc.vector.tensor_tensor(dst, pc[0:HD, :],
                                                rb[:, sub * 512:(sub + 1) * 512],
                                                op=ALU.mult)

                for i in range(len(units) + 1):
                    if i < len(units):
                        e_tiles[i] = epool.tile([P, ST, 1024], BF16, tag="e", name=f"e{i}")
                        c_tiles[i] = (cps.tile([HD + 1, 512], F32, tag="ca", name=f"ca{i}"),
                                      cps.tile([HD + 1, 512], F32, tag="cb", name=f"cb{i}"))
                    for kt in range(ST):
                        if i < len(units):
                            emit_scores_kt(i, kt)
                        if i > 0:
                            emit_ctx_kt(i - 1, kt)
                    if i > 0:
                        emit_ctx_tail(i - 1)
                        del e_tiles[i - 1]

            # ================= gather ctx across the TP group =================
            if STAGE >= 3:
                ag_in = dram.tile([HGD, S], BF16)
                nc.sync.dma_start(ag_in[0:P, :], ctxa_sb[:])
                nc.sync.dma_start(ag_in[P:HGD, :], ctxb_sb[:])
                ag_out = dram.tile([TP, HGD, S], BF16)
                nc.gpsimd.collective_compute(
                    "AllGather", ALU.bypass, replica_groups=GROUPS,
                    ins=[ag_in.opt()], outs=[ag_out.opt()],
                )

                with tc.tile_critical():
                    with nc.gpsimd.register("qo") as qo_reg:
                        nc.gpsimd.reg_load(qo_reg, qoff_sb[0:1, 0:1])
                        qoff_v = nc.gpsimd.snap(qo_reg)

                ctxg_sb = qkv.tile([P, KT, SQ], BF16)
                nc.gpsimd.dma_start(
                    ctxg_sb[:],
                    ag_out.rearrange("g d q -> (g d) q").rearrange("(kt p) q -> p kt q", p=P)[
                        :, :, bass.ds(qoff_v, SQ)],
                )

            # ================= out dense + residual + LayerNorm =================
            if STAGE < 4:
                with tc.tile_pool(name="dummy", bufs=1) as dpool:
                    for st4 in range(SQ // P):
                        d_sb = dpool.tile([P, H], F32, tag="d")
                        nc.vector.tensor_copy(d_sb[:], xres_sb[:, st4, :])
                        nc.sync.dma_start(out_d[st4 * P:(st4 + 1) * P, :], d_sb[:])
            with tc.tile_pool(name="ops", bufs=2, space="PSUM") as ops, \
                 tc.tile_pool(name="lnp", bufs=2) as lnp:
                for st4 in range(SQ // P if STAGE >= 4 else 0):
                    ps = ops.tile([P, H], F32, tag="od")
                    for kt in range(KT):
                        lhsT = ctxg_sb[:, kt, st4 * P:(st4 + 1) * P]
                        nc.tensor.matmul(ps[:, 0:512], lhsT, wo_sb[:, kt, 0:512],
                                         start=(kt == 0), stop=(kt == KT - 1))
                        nc.tensor.matmul(ps[:, 512:H], lhsT, wo_sb[:, kt, 512:H],
                                         start=(kt == 0), stop=(kt == KT - 1))
                    h_sb = lnp.tile([P, H], F32, tag="h")
                    nc.vector.tensor_tensor(h_sb[:], ps[:], xres_sb[:, st4, :], op=ALU.add)

                    if STAGE < 5:
                        nc.sync.dma_start(out_d[st4 * P:(st4 + 1) * P, :], h_sb[:])
                        continue

                    mu = lnp.tile([P, 1], F32, tag="mu")
                    nc.vector.reduce_sum(mu[:], h_sb[:], axis=AX.X)
                    nc.vector.tensor_scalar_mul(mu[:], mu[:], 1.0 / H)
                    hc = lnp.tile([P, H], F32, tag="hc")
                    nc.vector.tensor_scalar_sub(hc[:], h_sb[:], mu[:])
                    sq = lnp.tile([P, H], F32, tag="sq")
                    var = lnp.tile([P, 1], F32, tag="var")
                    if STAGE >= 7:
                        nc.vector.tensor_tensor_reduce(
                            out=sq[:], in0=hc[:], in1=hc[:], scale=1.0, scalar=0.0,
                            op0=ALU.mult, op1=ALU.add, accum_out=var[:])
                    else:
                        nc.vector.tensor_mul(sq[:], hc[:], hc[:])
                        nc.vector.reduce_sum(var[:], sq[:], axis=AX.X)
                    nc.vector.tensor_scalar_mul(var[:], var[:], 1.0 / H)
                    nc.vector.tensor_scalar_add(var[:], var[:], EPS)
                    # rstd = 1/sqrt(var), with one Newton step to fix ACT sqrt error
                    std0 = lnp.tile([P, 1], F32, tag="std0")
                    if STAGE >= 6:
                        nc.scalar.activation(std0[:], var[:], AF.Sqrt)
                    else:
                        nc.vector.tensor_copy(std0[:], var[:])
                    y0 = lnp.tile([P, 1], F32, tag="y0")
                    nc.vector.reciprocal(y0[:], std0[:])
                    t0 = lnp.tile([P, 1], F32, tag="t0")
                    nc.vector.tensor_tensor(t0[:], y0[:], y0[:], op=ALU.mult)
                    nc.vector.tensor_tensor(t0[:], t0[:], var[:], op=ALU.mult)
                    nc.vector.tensor_scalar_mul(t0[:], t0[:], -0.5)
                    nc.vector.tensor_scalar_add(t0[:], t0[:], 1.5)
                    rstd = lnp.tile([P, 1], F32, tag="rstd")
                    nc.vector.tensor_tensor(rstd[:], y0[:], t0[:], op=ALU.mult)

                    o_sb = lnp.tile([P, H], F32, tag="o")
                    nc.vector.tensor_scalar_mul(o_sb[:], hc[:], rstd[:])
                    nc.vector.tensor_tensor(o_sb[:], o_sb[:], lng_sb[:], op=ALU.mult)
                    nc.vector.tensor_tensor(o_sb[:], o_sb[:], lnb_sb[:], op=ALU.add)
                    nc.sync.dma_start(out_d[st4 * P:(st4 + 1) * P, :], o_sb[:])

    nc.compile()
    return nc


def _prep_inputs(inputs):
    hs = np.asarray(inputs["hidden_states"], dtype=np.float32)
    am = np.asarray(inputs["attention_mask"], dtype=np.float32)
    Wq = np.asarray(inputs["Wq"], dtype=np.float32)
    Wk = np.asarray(inputs["Wk"], dtype=np.float32)
    Wv = np.asarray(inputs["Wv"], dtype=np.float32)
    Wo = np.asarray(inputs["Wo"], dtype=np.float32)
    bq = np.asarray(inputs["bq"], dtype=np.float32)
    bk = np.asarray(inputs["bk"], dtype=np.float32)
    bv = np.asarray(inputs["bv"], dtype=np.float32)
    bo = np.asarray(inputs["bo"], dtype=np.float32)
    lng = np.asarray(inputs["ln_gamma"], dtype=np.float32)
    lnb = np.asarray(inputs["ln_beta"], dtype=np.float32)

    wo_bf = Wo.astype(ml_dtypes.bfloat16)
    lng_rep = np.ascontiguousarray(np.broadcast_to(lng, (P, H)))
    lnb_rep = np.ascontiguousarray(np.broadcast_to(lnb, (P, H)))

    in_maps = []
    for c in range(NCORES):
        b, g = c // TP, c % TP
        cs = slice(HGD * g, HGD * (g + 1))
        mk = np.where(am[b] >= 0, 0.0, BIGNEG).astype(np.float32)
        gqv = np.where(am[b] >= 0, 0.0, BIGPOS).astype(ml_dtypes.bfloat16)[None, :]
        in_maps.append({
            "xt": np.ascontiguousarray(hs[b].T).astype(ml_dtypes.bfloat16),
            "xres": np.ascontiguousarray(hs[b, SQ * g:SQ * (g + 1)] + bo),
            "wq": np.ascontiguousarray(Wq[:, cs]).astype(ml_dtypes.bfloat16),
            "wk": np.ascontiguousarray(Wk[:, cs]).astype(ml_dtypes.bfloat16),
            "wv": np.ascontiguousarray(Wv[:, cs]).astype(ml_dtypes.bfloat16),
            "bq": np.ascontiguousarray(bq[cs]),
            "bk": np.ascontiguousarray(bk[cs]),
            "bvr": np.ascontiguousarray(np.broadcast_to(bv[cs], (P, HGD))),
            "wo": np.ascontiguousarray(wo_bf),
            "mkb": mk,
            "gq": np.ascontiguousarray(gqv),
            "lng": lng_rep,
            "lnb": lnb_rep,
            "qoff": np.array([[SQ * g]], dtype=np.uint32),
        })
    return in_maps


def _run(inputs, trace=False, trace_cores=None):
    if "nc" not in _cache:
        _cache["nc"] = build()
    nc = _cache["nc"]
    in_maps = _prep_inputs(inputs)
    res = run_bass_kernel_spmd(
        nc, in_maps, list(range(NCORES)), trace=trace,
        trace_cores=trace_cores,
    )
    out = np.empty((B, S, H), dtype=np.float32)
    for c in range(NCORES):
        b, g = c // TP, c % TP
        out[b, SQ * g:SQ * (g + 1)] = res.results[c]["out"]
    return out, res


def kernel(**inputs) -> np.ndarray:
    out, _ = _run(inputs)
    return out
